# revision 31
# baseline (speedup 1.0000x reference)
"""Trainium2 Bass kernel for nn_DiagnoerMinBlcokScan (grouped 1D conv,
G=8 groups x FG=16 filters x J=8 channels, W=31 window, L=262144).

Strategy: data-parallel over L across 8 cores (no collectives; host slices
haloed shards). Inside each core the conv is phase-packed: output phases
o=0..7 fold into the matmul M dim (M = 16 filters x 8 phases = 128) and the
input is 8-phase deinterleaved so each of 5 "q" matmuls (contract = 64
(j,r) rows) reads a plain contiguous slice of SBUF. PSUM accumulates the 5
matmuls; result [128, n] tiles stream back to HBM. Group pairs share the
128-partition SBUF tiles (lower/upper 64 rows) so DMA runs full-width and
the PE can overlap row-disjoint matmuls.

Self-contained: hardcodes all shapes; host does the cheap boundary columns
(truncated-window semantics of the reference) and the phase re-interleave.
"""
import numpy as np

import concourse.bacc as bacc
import concourse.bass as bass
import concourse.mybir as mybir
from concourse.bass_utils import run_bass_kernel_spmd
from concourse.tile import TileContext
from concourse.tile_rust import add_dep_helper

D, L = 64, 262144
G, J, FG, W = 8, 8, 16, 31
PAD1 = 15
F = G * FG
NCORES = 8
LS = L // NCORES            # 32768 output cols per core
M = LS // 8                 # 4096 matmul free positions per core
MH = M + 4                  # deinterleaved cols incl. halo (n+q, q in 0..4)
NT = 512                    # matmul free-dim tile (one PSUM bank)
NTILES = M // NT            # 8

import os
_DT_NAME = os.environ.get("KERNEL_MM_DT", "bfloat16")
MM_DT = getattr(mybir.dt, _DT_NAME)
F32 = mybir.dt.float32
_NP_IN_DT = mybir.dt.np(MM_DT)

_cache = {}


def _build_bass(loop_n=None):
    """loop_n: if set, wrap the body in a HW loop (for slope timing)."""
    nc = bacc.Bacc()
    xb_h = nc.declare_dram_parameter("xb", [4, 128, MH], MM_DT, isOutput=False)
    w_h = nc.declare_dram_parameter("w", [128, 4 * 5 * 128], MM_DT, isOutput=False)
    y_h = nc.declare_dram_parameter("y", [G, 128, M], F32, isOutput=True)

    with TileContext(nc) as tc:
        with (
            tc.tile_pool(name="wpool", bufs=1) as wp,
            tc.tile_pool(name="xpool", bufs=2) as xp,
            tc.tile_pool(name="psum", bufs=5, space="PSUM") as pp,
            tc.tile_pool(name="psumb", bufs=1, space="PSUM") as pb,
            tc.tile_pool(name="psumd", bufs=1, space="PSUM") as pd,
            tc.tile_pool(name="ypool", bufs=64) as yp,
        ):
            # all weights resident: [128, 20*128] fp32 = 1.3 MB
            wt = wp.tile([128, 4 * 5 * 128], MM_DT)
            nc.sync.dma_start(out=wt[:], in_=w_h[:])

            # Engine (TPB) instructions may carry at most ONE sync wait, so
            # the structure below keeps every matmul/copy at <=1:
            #  - a throwaway matmul reading only `wt` absorbs the weight-DMA
            #    wait into the PE's vector clock (data-dep waits propagate;
            #    explicit-dep nops do not),
            #  - each pair's first group uses a dedicated PSUM slot whose
            #    release is a pair-old DVE tick the PE has already observed,
            #    so its matmul only waits on the xt DMA,
            #  - y staging tiles are never reused (bufs=64) so copies only
            #    wait on the PE.
            dummy = pd.tile([2, 2], F32)
            dmm = nc.tensor.matmul(dummy[:], wt[0:2, 0:2], wt[0:2, 0:2],
                                   start=True, stop=True)

            # HW For_i loops crash the exec unit in this environment; for
            # slope timing we statically unroll the body instead.
            for _ in range(loop_n or 1):
                _emit_body(nc, tc, wp, xp, pp, pb, pd, yp, wt, dmm, xb_h, y_h)
    nc.compile()
    return nc


def _emit_body(nc, tc, wp, xp, pp, pb, pd, yp, wt, dmm, xb_h, y_h):
    if True:
        if True:
            ncopy = 0
            for t in range(4):
                xt = xp.tile([128, MH], MM_DT)
                nc.sync.dma_start(out=xt[:], in_=xb_h[t])
                for i in range(NTILES):
                    n0 = NT * i
                    for half in range(2):
                        g = 2 * t + half
                        sl = slice(64 * half, 64 * half + 64)
                        boundary = (i == 0 and half == 0)
                        ps = (pb if boundary else pp).tile([128, NT], F32)
                        for q in range(5):
                            mm = nc.tensor.matmul(
                                ps[:],
                                wt[sl, (t * 5 + q) * 128:(t * 5 + q) * 128 + 128],
                                xt[sl, n0 + q: n0 + q + NT],
                                start=(q == 0),
                                stop=(q == 4),
                            )
                            if t == 0 and boundary and q == 0:
                                add_dep_helper(mm.ins, dmm.ins, sync=False,
                                               reason="order after wt gate")
                        yt = yp.tile([128, NT], F32)
                        if boundary or ncopy % 2 == 0:
                            nc.vector.tensor_copy(out=yt[:], in_=ps[:])
                        else:
                            nc.scalar.copy(out=yt[:], in_=ps[:])
                        ncopy += 1
                        nc.sync.dma_start(out=y_h[g, :, n0:n0 + NT], in_=yt[:])


def _host_prep(x, params, rel_idx):
    x2 = np.ascontiguousarray(np.asarray(x, dtype=np.float32)[np.asarray(rel_idx).reshape(-1)])
    x_pad = np.pad(x2, ((0, 0), (PAD1, 17)))                     # [64, L+32]
    xb_cores = []
    for c in range(NCORES):
        xs = x_pad[:, c * LS: c * LS + LS + 32]                  # [64, 8*MH]
        arr = xs.reshape(G, J, MH, 8).transpose(0, 1, 3, 2)      # [g, j, r, m]
        xb_cores.append(np.ascontiguousarray(
            arr.reshape(4, 128, MH)).astype(_NP_IN_DT))

    p = np.asarray(params, dtype=np.float32)
    wq = np.zeros((4, 5, 128, 128), dtype=np.float32)
    o_i = np.arange(8)
    r_i = np.arange(8)
    for g in range(G):
        t, half = g // 2, g % 2
        for q in range(5):
            w_mat = 8 * q + r_i[:, None] - o_i[None, :]          # [r, o]
            valid = (w_mat >= 0) & (w_mat <= 30)
            wm = np.where(valid, w_mat, 0)
            blk = p[g][:, :, wm] * valid[None, None]             # [f, j, r, o]
            blk = blk.transpose(1, 2, 0, 3).reshape(64, 128)     # [(j,r), (f,o)]
            wq[t, q, 64 * half:64 * half + 64, :] = blk
    # device layout: [128 partitions, (t, q, m) flattened]
    wq = np.ascontiguousarray(
        wq.transpose(2, 0, 1, 3).reshape(128, 4 * 5 * 128)).astype(_NP_IN_DT)
    return xb_cores, wq, x2, p


def _host_post(y_cores, x2, p):
    parts = [
        y.reshape(G, FG, 8, M).transpose(0, 1, 3, 2).reshape(G, FG, LS)
        for y in y_cores
    ]
    y_full = np.concatenate(parts, axis=2)                       # [G, FG, L]

    xg = x2.reshape(G, J, L)
    pl = np.einsum("gjw,gfjw->gfw", xg[:, :, :W], p)
    left_c = np.cumsum(pl, axis=-1)
    y_full[:, :, :PAD1] = left_c[:, :, W - PAD1 - 1: W - 1]
    pr = np.einsum("gjw,gfjw->gfw", xg[:, :, L - W:], p)
    right_c = np.cumsum(pr[:, :, ::-1], axis=-1)[:, :, ::-1]
    n_right = W - 1 - PAD1
    y_full[:, :, L - n_right:] = right_c[:, :, 1: W - PAD1]
    return np.ascontiguousarray(y_full.reshape(F * L, 1), dtype=np.float32)


def kernel(x, params, rel_idx, _trace=False, _trace_out=None):
    if "nc" not in _cache:
        _cache["nc"] = _build_bass()
    nc = _cache["nc"]

    xb_cores, wq, x2, p = _host_prep(x, params, rel_idx)
    in_maps = [{"xb": xb_cores[c], "w": wq} for c in range(NCORES)]
    res = run_bass_kernel_spmd(
        nc, in_maps, list(range(NCORES)),
        trace=_trace, **({"tmpdir": _trace_out} if _trace_out else {}),
    )
    if _trace_out is not None:
        _cache["last"] = res
    y_cores = [np.asarray(res.results[c]["y"]) for c in range(NCORES)]
    return _host_post(y_cores, x2, p)


# revision 41
# speedup vs baseline: 8.6265x; 8.6265x over previous
"""Trainium2 Bass kernel for nn_DiagnoerMinBlcokScan (grouped 1D conv,
G=8 groups x FG=16 filters x J=8 channels, W=31 window, L=262144).

Strategy: data-parallel over L across 8 cores (no collectives; host slices
haloed shards). Inside each core the conv is phase-packed: output phases
o=0..7 fold into the matmul M dim (M = 16 filters x 8 phases = 128) and the
input is 8-phase deinterleaved so each of 5 "q" matmuls (contract = 64
(j,r) rows) reads a plain contiguous slice of SBUF. PSUM accumulates the 5
matmuls; result [128, n] tiles stream back to HBM. Group pairs share the
128-partition SBUF tiles (lower/upper 64 rows) so DMA runs full-width and
the PE can overlap row-disjoint matmuls.

Self-contained: hardcodes all shapes; host does the cheap boundary columns
(truncated-window semantics of the reference) and the phase re-interleave.
"""
import numpy as np

import concourse.bacc as bacc
import concourse.bass as bass
import concourse.mybir as mybir
from concourse.bass_utils import run_bass_kernel_spmd
from concourse.tile import TileContext
from concourse.tile_rust import add_dep_helper

D, L = 64, 262144
G, J, FG, W = 8, 8, 16, 31
PAD1 = 15
F = G * FG
NCORES = 8
LS = L // NCORES            # 32768 output cols per core
M = LS // 8                 # 4096 matmul free positions per core
MH = M + 4                  # deinterleaved cols incl. halo (n+q, q in 0..4)
NT = 512                    # matmul free-dim tile (one PSUM bank)
NTILES = M // NT            # 8

import os
# float16: full-rate PE streaming (1 col/cycle) with 10-bit mantissa;
# fp32 PSUM accumulation. Measured rel err ~3e-4 end to end.
_DT_NAME = os.environ.get("KERNEL_MM_DT", "float16")
MM_DT = getattr(mybir.dt, _DT_NAME)
F32 = mybir.dt.float32
_NP_IN_DT = mybir.dt.np(MM_DT)

_cache = {}


def _build_bass(loop_n=None, internal_io=False):
    """loop_n: if set, emit the body N times (for slope timing).
    internal_io: DRAM traffic goes to internal scratch (timing-only builds,
    avoids shipping real data through the PJRT tunnel)."""
    nc = bacc.Bacc()
    if internal_io:
        xb_h = nc.dram_tensor("xb_i", [4, 128, MH], MM_DT)
        w_h = nc.dram_tensor("w_i", [128, 4 * 5 * 128], MM_DT)
        y_h = nc.dram_tensor("y_i", [G, 128, M], F32)
        sent_in = nc.declare_dram_parameter("s_in", [8, 4], F32, isOutput=False)
        sent_out = nc.declare_dram_parameter("s_out", [8, 4], F32, isOutput=True)
        nc._sentinel = (sent_in, sent_out)
    else:
        xb_h = nc.declare_dram_parameter("xb", [4, 128, MH], MM_DT, isOutput=False)
        w_h = nc.declare_dram_parameter("w", [128, 4 * 5 * 128], MM_DT, isOutput=False)
        y_h = nc.declare_dram_parameter("y", [G, 128, M], F32, isOutput=True)

    with TileContext(nc) as tc:
        with (
            tc.tile_pool(name="wpool", bufs=1) as wp,
            tc.tile_pool(name="xpool", bufs=2) as xp,
            tc.tile_pool(name="psum", bufs=5, space="PSUM") as pp,
            tc.tile_pool(name="psumb", bufs=1, space="PSUM") as pb,
            tc.tile_pool(name="psumd", bufs=1, space="PSUM") as pd,
            tc.tile_pool(name="ypool", bufs=64) as yp,
        ):
            # all weights resident: [128, 20*128] fp32 = 1.3 MB
            wt = wp.tile([128, 4 * 5 * 128], MM_DT)
            nc.sync.dma_start(out=wt[:], in_=w_h[:])
            if internal_io:
                si, so = nc._sentinel
                nc.sync.dma_start(out=so[:], in_=si[:])

            # Engine (TPB) instructions may carry at most ONE sync wait, so
            # the structure below keeps every matmul/copy at <=1:
            #  - a throwaway matmul reading only `wt` absorbs the weight-DMA
            #    wait into the PE's vector clock (data-dep waits propagate;
            #    explicit-dep nops do not),
            #  - each pair's first group uses a dedicated PSUM slot whose
            #    release is a pair-old DVE tick the PE has already observed,
            #    so its matmul only waits on the xt DMA,
            #  - y staging tiles are never reused (bufs=64) so copies only
            #    wait on the PE.
            dummy = pd.tile([2, 2], F32)
            dmm = nc.tensor.matmul(dummy[:], wt[0:2, 0:2], wt[0:2, 0:2],
                                   start=True, stop=True)

            # HW For_i loops crash the exec unit in this environment; for
            # slope timing we statically unroll the body instead.
            for _ in range(loop_n or 1):
                _emit_body(nc, tc, wp, xp, pp, pb, pd, yp, wt, dmm, xb_h, y_h)
    nc.compile()
    return nc


def _emit_body(nc, tc, wp, xp, pp, pb, pd, yp, wt, dmm, xb_h, y_h):
    if True:
        if True:
            ncopy = 0
            for t in range(4):
                xt = xp.tile([128, MH], MM_DT)
                nc.sync.dma_start(out=xt[:], in_=xb_h[t])
                for i in range(NTILES):
                    n0 = NT * i
                    for half in range(2):
                        g = 2 * t + half
                        sl = slice(64 * half, 64 * half + 64)
                        boundary = (i == 0 and half == 0)
                        ps = (pb if boundary else pp).tile([128, NT], F32)
                        for q in range(5):
                            mm = nc.tensor.matmul(
                                ps[:],
                                wt[sl, (t * 5 + q) * 128:(t * 5 + q) * 128 + 128],
                                xt[sl, n0 + q: n0 + q + NT],
                                start=(q == 0),
                                stop=(q == 4),
                            )
                            if t == 0 and boundary and q == 0:
                                add_dep_helper(mm.ins, dmm.ins, sync=False,
                                               reason="order after wt gate")
                        yt = yp.tile([128, NT], F32)
                        if boundary or ncopy % 2 == 0:
                            nc.vector.tensor_copy(out=yt[:], in_=ps[:])
                        else:
                            nc.scalar.copy(out=yt[:], in_=ps[:])
                        ncopy += 1
                        nc.sync.dma_start(out=y_h[g, :, n0:n0 + NT], in_=yt[:])


def _build_bass_v5(loop_n=None, internal_io=False):
    """Production kernel: 3-matmul decomposition. q-pairs (0,2) and (1,3)
    pack into contract-128 matmuls against a [xr ; xr shifted +2 cols] SBUF
    tile whose upper half is built on-chip by a DVE copy; q=4 rides alone at
    contract 64. All 8 group input DMAs + duplicates are hoisted to the
    front (x tiles fully resident) so input DMAs never queue behind output
    DMAs; weights stream in per-group chunks so the PE starts on group 0
    after ~100 KB of DMA. Cost model: 64.2 us/core vs 62.4 us DMA floor."""
    nc = bacc.Bacc()
    if internal_io:
        xr_h = nc.dram_tensor("xr_i", [G, 64, MH], MM_DT)
        w_h = nc.dram_tensor("w_i", [128, G * 3 * 128], MM_DT)
        y_h = nc.dram_tensor("y_i", [G, 128, M], F32)
        sent_in = nc.declare_dram_parameter("s_in", [8, 4], F32, isOutput=False)
        sent_out = nc.declare_dram_parameter("s_out", [8, 4], F32, isOutput=True)
    else:
        xr_h = nc.declare_dram_parameter("xr", [G, 64, MH], MM_DT, isOutput=False)
        w_h = nc.declare_dram_parameter("w", [128, G * 3 * 128], MM_DT, isOutput=False)
        y_h = nc.declare_dram_parameter("y", [G, 128, M], F32, isOutput=True)

    with TileContext(nc) as tc:
        with (
            tc.tile_pool(name="wpool", bufs=1) as wp,
            tc.tile_pool(name="xpool", bufs=8) as xp,
            tc.tile_pool(name="psum", bufs=6, space="PSUM") as pp,
            tc.tile_pool(name="psumd", bufs=1, space="PSUM") as pd,
            tc.tile_pool(name="ypool", bufs=36) as yp,
        ):
            wt = wp.tile([128, G * 3 * 128], MM_DT)
            xgs = []
            xg0 = xp.tile([128, MH], MM_DT)
            nc.sync.dma_start(out=xg0[0:64, :], in_=xr_h[0])
            nc.sync.dma_start(out=wt[:, 0:384], in_=w_h[:, 0:384])
            nc.vector.tensor_copy(out=xg0[64:128, 0:MH - 2], in_=xg0[0:64, 2:MH])
            dummy = pd.tile([2, 2], F32)
            nc.tensor.matmul(dummy[:], wt[0:2, 0:2], wt[0:2, 0:2],
                             start=True, stop=True)
            xgs.append(xg0)
            if internal_io:
                nc.sync.dma_start(out=sent_out[:], in_=sent_in[:])
            for g in range(1, G):
                xg = xp.tile([128, MH], MM_DT)
                nc.sync.dma_start(out=xg[0:64, :], in_=xr_h[g])
                nc.sync.dma_start(out=wt[:, g * 384:(g + 1) * 384],
                                  in_=w_h[:, g * 384:(g + 1) * 384])
                xgs.append(xg)
            for g in range(1, G):
                nc.vector.tensor_copy(out=xgs[g][64:128, 0:MH - 2],
                                      in_=xgs[g][0:64, 2:MH])
            for _ in range(loop_n or 1):
                ncopy = 0
                for g in range(G):
                    xg = xgs[g]
                    for i in range(NTILES):
                        n0 = NT * i
                        ps = pp.tile([128, NT], F32)
                        wof = g * 3 * 128
                        nc.tensor.matmul(ps[:], wt[:, wof:wof + 128],
                                         xg[:, n0:n0 + NT],
                                         start=True, stop=False)
                        nc.tensor.matmul(ps[:], wt[:, wof + 128:wof + 256],
                                         xg[:, n0 + 1:n0 + 1 + NT],
                                         start=False, stop=False)
                        nc.tensor.matmul(ps[:], wt[0:64, wof + 256:wof + 384],
                                         xg[0:64, n0 + 4:n0 + 4 + NT],
                                         start=False, stop=True)
                        yt = yp.tile([128, NT], F32)
                        if ncopy % 2 == 0:
                            nc.vector.tensor_copy(out=yt[:], in_=ps[:])
                        else:
                            nc.scalar.copy(out=yt[:], in_=ps[:])
                        ncopy += 1
                        nc.sync.dma_start(out=y_h[g, :, n0:n0 + NT], in_=yt[:])
    nc.compile()
    return nc


def _build_bass_v3(loop_n=None, internal_io=False):
    """3-matmul variant: q-pairs (0,2) and (1,3) packed into contract-128
    matmuls against a [xr ; xr shifted +2] SBUF tile built on-chip by
    gpsimd; q=4 rides alone at contract 64. PE streams 3x512 cols per
    group-tile instead of 5x512."""
    nc = bacc.Bacc()
    if internal_io:
        xr_h = nc.dram_tensor("xr_i", [G, 64, MH], MM_DT)
        w_h = nc.dram_tensor("w_i", [128, G * 3 * 128], MM_DT)
        y_h = nc.dram_tensor("y_i", [G, 128, M], F32)
        sent_in = nc.declare_dram_parameter("s_in", [8, 4], F32, isOutput=False)
        sent_out = nc.declare_dram_parameter("s_out", [8, 4], F32, isOutput=True)
    else:
        xr_h = nc.declare_dram_parameter("xr", [G, 64, MH], MM_DT, isOutput=False)
        w_h = nc.declare_dram_parameter("w", [128, G * 3 * 128], MM_DT, isOutput=False)
        y_h = nc.declare_dram_parameter("y", [G, 128, M], F32, isOutput=True)

    with TileContext(nc) as tc:
        with (
            tc.tile_pool(name="wpool", bufs=1) as wp,
            tc.tile_pool(name="xpool", bufs=3) as xp,
            tc.tile_pool(name="psum", bufs=6, space="PSUM") as pp,
            tc.tile_pool(name="psumd", bufs=1, space="PSUM") as pd,
            tc.tile_pool(name="ypool", bufs=64) as yp,
        ):
            wt = wp.tile([128, G * 3 * 128], MM_DT)
            nc.sync.dma_start(out=wt[:], in_=w_h[:])
            if internal_io:
                nc.sync.dma_start(out=sent_out[:], in_=sent_in[:])
            dummy = pd.tile([2, 2], F32)
            nc.tensor.matmul(dummy[:], wt[0:2, 0:2], wt[0:2, 0:2],
                             start=True, stop=True)
            for _ in range(loop_n or 1):
                ncopy = 0
                for g in range(G):
                    xg = xp.tile([128, MH], MM_DT)
                    nc.sync.dma_start(out=xg[0:64, :], in_=xr_h[g])
                    nc.gpsimd.tensor_copy(out=xg[64:128, 0:MH - 2],
                                          in_=xg[0:64, 2:MH])
                    for i in range(NTILES):
                        n0 = NT * i
                        ps = pp.tile([128, NT], F32)
                        wof = g * 3 * 128
                        nc.tensor.matmul(ps[:], wt[:, wof:wof + 128],
                                         xg[:, n0:n0 + NT],
                                         start=True, stop=False)
                        nc.tensor.matmul(ps[:], wt[:, wof + 128:wof + 256],
                                         xg[:, n0 + 1:n0 + 1 + NT],
                                         start=False, stop=False)
                        nc.tensor.matmul(ps[:], wt[0:64, wof + 256:wof + 384],
                                         xg[0:64, n0 + 4:n0 + 4 + NT],
                                         start=False, stop=True)
                        yt = yp.tile([128, NT], F32)
                        if ncopy % 2 == 0:
                            nc.vector.tensor_copy(out=yt[:], in_=ps[:])
                        else:
                            nc.scalar.copy(out=yt[:], in_=ps[:])
                        ncopy += 1
                        nc.sync.dma_start(out=y_h[g, :, n0:n0 + NT], in_=yt[:])
    nc.compile()
    return nc


def _host_prep_v3(x, params, rel_idx):
    x2 = np.ascontiguousarray(np.asarray(x, dtype=np.float32)[np.asarray(rel_idx).reshape(-1)])
    x_pad = np.pad(x2, ((0, 0), (PAD1, 17)))
    xr_cores = []
    for c in range(NCORES):
        xs = x_pad[:, c * LS: c * LS + LS + 32]
        arr = xs.reshape(G, J, MH, 8).transpose(0, 1, 3, 2)      # [g, j, r, m]
        xr_cores.append(arr.reshape(G, 64, MH).astype(_NP_IN_DT))

    p = np.asarray(params, dtype=np.float32)
    o_i = np.arange(8)
    r_i = np.arange(8)
    W5 = np.zeros((G, 5, 64, 128), dtype=np.float32)
    for g in range(G):
        for q in range(5):
            w_mat = 8 * q + r_i[:, None] - o_i[None, :]
            valid = (w_mat >= 0) & (w_mat <= 30)
            wm = np.where(valid, w_mat, 0)
            blk = p[g][:, :, wm] * valid[None, None]
            W5[g, q] = blk.transpose(1, 2, 0, 3).reshape(64, 128)
    # [128, G*3*128]: per group three lhsT mats A=[q0;q2], B=[q1;q3], C=[q4;0]
    wq3 = np.zeros((G, 3, 128, 128), dtype=np.float32)
    wq3[:, 0, 0:64], wq3[:, 0, 64:128] = W5[:, 0], W5[:, 2]
    wq3[:, 1, 0:64], wq3[:, 1, 64:128] = W5[:, 1], W5[:, 3]
    wq3[:, 2, 0:64] = W5[:, 4]
    wq3 = np.ascontiguousarray(
        wq3.transpose(2, 0, 1, 3).reshape(128, G * 3 * 128)).astype(_NP_IN_DT)
    return xr_cores, wq3, x2, p


def _host_prep(x, params, rel_idx):
    x2 = np.ascontiguousarray(np.asarray(x, dtype=np.float32)[np.asarray(rel_idx).reshape(-1)])
    x_pad = np.pad(x2, ((0, 0), (PAD1, 17)))                     # [64, L+32]
    xb_cores = []
    for c in range(NCORES):
        xs = x_pad[:, c * LS: c * LS + LS + 32]                  # [64, 8*MH]
        arr = xs.reshape(G, J, MH, 8).transpose(0, 1, 3, 2)      # [g, j, r, m]
        xb_cores.append(np.ascontiguousarray(
            arr.reshape(4, 128, MH)).astype(_NP_IN_DT))

    p = np.asarray(params, dtype=np.float32)
    wq = np.zeros((4, 5, 128, 128), dtype=np.float32)
    o_i = np.arange(8)
    r_i = np.arange(8)
    for g in range(G):
        t, half = g // 2, g % 2
        for q in range(5):
            w_mat = 8 * q + r_i[:, None] - o_i[None, :]          # [r, o]
            valid = (w_mat >= 0) & (w_mat <= 30)
            wm = np.where(valid, w_mat, 0)
            blk = p[g][:, :, wm] * valid[None, None]             # [f, j, r, o]
            blk = blk.transpose(1, 2, 0, 3).reshape(64, 128)     # [(j,r), (f,o)]
            wq[t, q, 64 * half:64 * half + 64, :] = blk
    # device layout: [128 partitions, (t, q, m) flattened]
    wq = np.ascontiguousarray(
        wq.transpose(2, 0, 1, 3).reshape(128, 4 * 5 * 128)).astype(_NP_IN_DT)
    return xb_cores, wq, x2, p


def _host_post(y_cores, x2, p):
    parts = [
        y.reshape(G, FG, 8, M).transpose(0, 1, 3, 2).reshape(G, FG, LS)
        for y in y_cores
    ]
    y_full = np.concatenate(parts, axis=2)                       # [G, FG, L]

    xg = x2.reshape(G, J, L)
    pl = np.einsum("gjw,gfjw->gfw", xg[:, :, :W], p)
    left_c = np.cumsum(pl, axis=-1)
    y_full[:, :, :PAD1] = left_c[:, :, W - PAD1 - 1: W - 1]
    pr = np.einsum("gjw,gfjw->gfw", xg[:, :, L - W:], p)
    right_c = np.cumsum(pr[:, :, ::-1], axis=-1)[:, :, ::-1]
    n_right = W - 1 - PAD1
    y_full[:, :, L - n_right:] = right_c[:, :, 1: W - PAD1]
    return np.ascontiguousarray(y_full.reshape(F * L, 1), dtype=np.float32)


def _build_fn(nc):
    """Jitted 8-core shard_map executor for the compiled Bass module.
    Zero-init output buffers are created on device (no host upload)."""
    import jax
    import jax.numpy as jnp
    from jax.sharding import Mesh, PartitionSpec
    from jax.experimental.shard_map import shard_map
    from concourse.bass2jax import (
        _bass_exec_p, install_neuronx_cc_hook, partition_id_tensor)

    install_neuronx_cc_hook()
    partition_name = nc.partition_id_tensor.name if nc.partition_id_tensor else None
    in_names, out_names, out_avals = [], [], []
    for alloc in nc.m.functions[0].allocations:
        if not isinstance(alloc, mybir.MemoryLocationSet):
            continue
        name = alloc.memorylocations[0].name
        if alloc.kind == "ExternalInput":
            if name != partition_name:
                in_names.append(name)
        elif alloc.kind == "ExternalOutput":
            out_names.append(name)
            out_avals.append(jax.core.ShapedArray(
                tuple(alloc.tensor_shape), mybir.dt.np(alloc.dtype)))
    all_names = list(in_names) + list(out_names)
    if partition_name is not None:
        all_names.append(partition_name)

    def _body(*args):
        operands = list(args)
        if partition_name is not None:
            operands.append(partition_id_tensor())
        return tuple(_bass_exec_p.bind(
            *operands,
            out_avals=tuple(out_avals),
            in_names=tuple(all_names),
            out_names=tuple(out_names),
            lowering_input_output_aliases=(),
            sim_require_finite=True,
            sim_require_nnan=True,
            nc=nc,
        ))

    devices = jax.devices()[:NCORES]
    mesh = Mesh(np.asarray(devices), ("core",))
    nin = len(in_names) + len(out_avals)
    fn = jax.jit(shard_map(
        _body, mesh=mesh,
        in_specs=(PartitionSpec("core"),) * nin,
        out_specs=(PartitionSpec("core"),) * len(out_names),
        check_rep=False))
    # zero output buffers, materialized directly on device (no upload)
    sh = jax.sharding.NamedSharding(mesh, PartitionSpec("core"))
    zeros = [
        jax.jit(lambda av=av: jnp.zeros((NCORES * av.shape[0],) + av.shape[1:],
                                        av.dtype), out_shardings=sh)()
        for av in out_avals
    ]
    return fn, in_names, out_names, zeros


def kernel(x, params, rel_idx, _trace=False, _trace_out=None):
    if "nc" not in _cache:
        _cache["nc"] = _build_bass_v5()
        _cache["fn"] = _build_fn(_cache["nc"])
    nc = _cache["nc"]

    xr_cores, wq3, x2, p = _host_prep_v3(x, params, rel_idx)
    try:
        fn, in_names, out_names, zeros = _cache["fn"]
        per = {"xr": np.stack(xr_cores),
               "w": np.broadcast_to(wq3, (NCORES,) + wq3.shape)}
        concat = [np.ascontiguousarray(per[nm].reshape(
            NCORES * per[nm].shape[1], *per[nm].shape[2:])) for nm in in_names]
        outs = fn(*concat, *zeros)
        yi = out_names.index("y")
        y_all = np.asarray(outs[yi]).reshape(NCORES, G, 128, M)
        y_cores = [y_all[c] for c in range(NCORES)]
    except Exception:
        # fallback: reference SPMD runner
        in_maps = [{"xr": xr_cores[c], "w": wq3} for c in range(NCORES)]
        res = run_bass_kernel_spmd(nc, in_maps, list(range(NCORES)))
        y_cores = [np.asarray(res.results[c]["y"]) for c in range(NCORES)]
    return _host_post(y_cores, x2, p)


# revision 45
# speedup vs baseline: 9.2599x; 1.0734x over previous
"""Trainium2 Bass kernel for nn_DiagnoerMinBlcokScan (grouped 1D conv,
G=8 groups x FG=16 filters x J=8 channels, W=31 window, L=262144).

Strategy: data-parallel over L across 8 cores (no collectives; host slices
haloed shards). Inside each core the conv is phase-packed: output phases
o=0..7 fold into the matmul M dim (M = 16 filters x 8 phases = 128) and the
input is 8-phase deinterleaved so each of 5 "q" matmuls (contract = 64
(j,r) rows) reads a plain contiguous slice of SBUF. PSUM accumulates the 5
matmuls; result [128, n] tiles stream back to HBM. Group pairs share the
128-partition SBUF tiles (lower/upper 64 rows) so DMA runs full-width and
the PE can overlap row-disjoint matmuls.

Self-contained: hardcodes all shapes; host does the cheap boundary columns
(truncated-window semantics of the reference) and the phase re-interleave.
"""
import numpy as np

import concourse.bacc as bacc
import concourse.bass as bass
import concourse.mybir as mybir
from concourse.bass_utils import run_bass_kernel_spmd
from concourse.tile import TileContext
from concourse.tile_rust import add_dep_helper

D, L = 64, 262144
G, J, FG, W = 8, 8, 16, 31
PAD1 = 15
F = G * FG
NCORES = 8
LS = L // NCORES            # 32768 output cols per core
M = LS // 8                 # 4096 matmul free positions per core
MH = M + 4                  # deinterleaved cols incl. halo (n+q, q in 0..4)
NT = 512                    # matmul free-dim tile (one PSUM bank)
NTILES = M // NT            # 8

import os
# float16: full-rate PE streaming (1 col/cycle) with 10-bit mantissa;
# fp32 PSUM accumulation. Measured rel err ~3e-4 end to end.
_DT_NAME = os.environ.get("KERNEL_MM_DT", "float16")
MM_DT = getattr(mybir.dt, _DT_NAME)
F32 = mybir.dt.float32
# Output leaves the device as fp16 (halves the dominant DMA stream; host
# upcasts). Adds ~1.3e-4 L2 error on top of the fp16-compute 3e-4.
Y_DT = getattr(mybir.dt, os.environ.get("KERNEL_Y_DT", "float16"))
_NP_IN_DT = mybir.dt.np(MM_DT)
_NP_Y_DT = mybir.dt.np(Y_DT)

_cache = {}


def _build_bass(loop_n=None, internal_io=False):
    """loop_n: if set, emit the body N times (for slope timing).
    internal_io: DRAM traffic goes to internal scratch (timing-only builds,
    avoids shipping real data through the PJRT tunnel)."""
    nc = bacc.Bacc()
    if internal_io:
        xb_h = nc.dram_tensor("xb_i", [4, 128, MH], MM_DT)
        w_h = nc.dram_tensor("w_i", [128, 4 * 5 * 128], MM_DT)
        y_h = nc.dram_tensor("y_i", [G, 128, M], F32)
        sent_in = nc.declare_dram_parameter("s_in", [8, 4], F32, isOutput=False)
        sent_out = nc.declare_dram_parameter("s_out", [8, 4], F32, isOutput=True)
        nc._sentinel = (sent_in, sent_out)
    else:
        xb_h = nc.declare_dram_parameter("xb", [4, 128, MH], MM_DT, isOutput=False)
        w_h = nc.declare_dram_parameter("w", [128, 4 * 5 * 128], MM_DT, isOutput=False)
        y_h = nc.declare_dram_parameter("y", [G, 128, M], F32, isOutput=True)

    with TileContext(nc) as tc:
        with (
            tc.tile_pool(name="wpool", bufs=1) as wp,
            tc.tile_pool(name="xpool", bufs=2) as xp,
            tc.tile_pool(name="psum", bufs=5, space="PSUM") as pp,
            tc.tile_pool(name="psumb", bufs=1, space="PSUM") as pb,
            tc.tile_pool(name="psumd", bufs=1, space="PSUM") as pd,
            tc.tile_pool(name="ypool", bufs=64) as yp,
        ):
            # all weights resident: [128, 20*128] fp32 = 1.3 MB
            wt = wp.tile([128, 4 * 5 * 128], MM_DT)
            nc.sync.dma_start(out=wt[:], in_=w_h[:])
            if internal_io:
                si, so = nc._sentinel
                nc.sync.dma_start(out=so[:], in_=si[:])

            # Engine (TPB) instructions may carry at most ONE sync wait, so
            # the structure below keeps every matmul/copy at <=1:
            #  - a throwaway matmul reading only `wt` absorbs the weight-DMA
            #    wait into the PE's vector clock (data-dep waits propagate;
            #    explicit-dep nops do not),
            #  - each pair's first group uses a dedicated PSUM slot whose
            #    release is a pair-old DVE tick the PE has already observed,
            #    so its matmul only waits on the xt DMA,
            #  - y staging tiles are never reused (bufs=64) so copies only
            #    wait on the PE.
            dummy = pd.tile([2, 2], F32)
            dmm = nc.tensor.matmul(dummy[:], wt[0:2, 0:2], wt[0:2, 0:2],
                                   start=True, stop=True)

            # HW For_i loops crash the exec unit in this environment; for
            # slope timing we statically unroll the body instead.
            for _ in range(loop_n or 1):
                _emit_body(nc, tc, wp, xp, pp, pb, pd, yp, wt, dmm, xb_h, y_h)
    nc.compile()
    return nc


def _emit_body(nc, tc, wp, xp, pp, pb, pd, yp, wt, dmm, xb_h, y_h):
    if True:
        if True:
            ncopy = 0
            for t in range(4):
                xt = xp.tile([128, MH], MM_DT)
                nc.sync.dma_start(out=xt[:], in_=xb_h[t])
                for i in range(NTILES):
                    n0 = NT * i
                    for half in range(2):
                        g = 2 * t + half
                        sl = slice(64 * half, 64 * half + 64)
                        boundary = (i == 0 and half == 0)
                        ps = (pb if boundary else pp).tile([128, NT], F32)
                        for q in range(5):
                            mm = nc.tensor.matmul(
                                ps[:],
                                wt[sl, (t * 5 + q) * 128:(t * 5 + q) * 128 + 128],
                                xt[sl, n0 + q: n0 + q + NT],
                                start=(q == 0),
                                stop=(q == 4),
                            )
                            if t == 0 and boundary and q == 0:
                                add_dep_helper(mm.ins, dmm.ins, sync=False,
                                               reason="order after wt gate")
                        yt = yp.tile([128, NT], F32)
                        if boundary or ncopy % 2 == 0:
                            nc.vector.tensor_copy(out=yt[:], in_=ps[:])
                        else:
                            nc.scalar.copy(out=yt[:], in_=ps[:])
                        ncopy += 1
                        nc.sync.dma_start(out=y_h[g, :, n0:n0 + NT], in_=yt[:])


def _build_bass_v5(loop_n=None, internal_io=False):
    """Production kernel: 3-matmul decomposition. q-pairs (0,2) and (1,3)
    pack into contract-128 matmuls against a [xr ; xr shifted +2 cols] SBUF
    tile whose upper half is built on-chip by a DVE copy; q=4 rides alone at
    contract 64. All 8 group input DMAs + duplicates are hoisted to the
    front (x tiles fully resident) so input DMAs never queue behind output
    DMAs; weights stream in per-group chunks so the PE starts on group 0
    after ~100 KB of DMA. Cost model: 64.2 us/core vs 62.4 us DMA floor."""
    nc = bacc.Bacc()
    if internal_io:
        xr_h = nc.dram_tensor("xr_i", [G, 64, MH], MM_DT)
        w_h = nc.dram_tensor("w_i", [128, G * 3 * 128], MM_DT)
        y_h = nc.dram_tensor("y_i", [G, 128, M], Y_DT)
        sent_in = nc.declare_dram_parameter("s_in", [8, 4], F32, isOutput=False)
        sent_out = nc.declare_dram_parameter("s_out", [8, 4], F32, isOutput=True)
    else:
        xr_h = nc.declare_dram_parameter("xr", [G, 64, MH], MM_DT, isOutput=False)
        w_h = nc.declare_dram_parameter("w", [128, G * 3 * 128], MM_DT, isOutput=False)
        y_h = nc.declare_dram_parameter("y", [G, 128, M], Y_DT, isOutput=True)

    with TileContext(nc) as tc:
        with (
            tc.tile_pool(name="wpool", bufs=1) as wp,
            tc.tile_pool(name="xpool", bufs=8) as xp,
            tc.tile_pool(name="psum", bufs=6, space="PSUM") as pp,
            tc.tile_pool(name="psumd", bufs=1, space="PSUM") as pd,
            tc.tile_pool(name="ypool", bufs=36) as yp,
        ):
            wt = wp.tile([128, G * 3 * 128], MM_DT)
            xgs = []
            xg0 = xp.tile([128, MH], MM_DT)
            nc.sync.dma_start(out=xg0[0:64, :], in_=xr_h[0])
            nc.sync.dma_start(out=wt[:, 0:384], in_=w_h[:, 0:384])
            nc.vector.tensor_copy(out=xg0[64:128, 0:MH - 2], in_=xg0[0:64, 2:MH])
            dummy = pd.tile([2, 2], F32)
            nc.tensor.matmul(dummy[:], wt[0:2, 0:2], wt[0:2, 0:2],
                             start=True, stop=True)
            xgs.append(xg0)
            if internal_io:
                nc.sync.dma_start(out=sent_out[:], in_=sent_in[:])
            for g in range(1, G):
                xg = xp.tile([128, MH], MM_DT)
                nc.sync.dma_start(out=xg[0:64, :], in_=xr_h[g])
                nc.sync.dma_start(out=wt[:, g * 384:(g + 1) * 384],
                                  in_=w_h[:, g * 384:(g + 1) * 384])
                xgs.append(xg)
            for g in range(1, G):
                nc.vector.tensor_copy(out=xgs[g][64:128, 0:MH - 2],
                                      in_=xgs[g][0:64, 2:MH])
            for _ in range(loop_n or 1):
                ncopy = 0
                for g in range(G):
                    xg = xgs[g]
                    for i in range(NTILES):
                        n0 = NT * i
                        ps = pp.tile([128, NT], F32)
                        wof = g * 3 * 128
                        nc.tensor.matmul(ps[:], wt[:, wof:wof + 128],
                                         xg[:, n0:n0 + NT],
                                         start=True, stop=False)
                        nc.tensor.matmul(ps[:], wt[:, wof + 128:wof + 256],
                                         xg[:, n0 + 1:n0 + 1 + NT],
                                         start=False, stop=False)
                        nc.tensor.matmul(ps[:], wt[0:64, wof + 256:wof + 384],
                                         xg[0:64, n0 + 4:n0 + 4 + NT],
                                         start=False, stop=True)
                        yt = yp.tile([128, NT], Y_DT)
                        if ncopy % 2 == 0:
                            nc.vector.tensor_copy(out=yt[:], in_=ps[:])
                        else:
                            nc.scalar.copy(out=yt[:], in_=ps[:])
                        ncopy += 1
                        nc.sync.dma_start(out=y_h[g, :, n0:n0 + NT], in_=yt[:])
    nc.compile()
    return nc


def _build_bass_v3(loop_n=None, internal_io=False):
    """3-matmul variant: q-pairs (0,2) and (1,3) packed into contract-128
    matmuls against a [xr ; xr shifted +2] SBUF tile built on-chip by
    gpsimd; q=4 rides alone at contract 64. PE streams 3x512 cols per
    group-tile instead of 5x512."""
    nc = bacc.Bacc()
    if internal_io:
        xr_h = nc.dram_tensor("xr_i", [G, 64, MH], MM_DT)
        w_h = nc.dram_tensor("w_i", [128, G * 3 * 128], MM_DT)
        y_h = nc.dram_tensor("y_i", [G, 128, M], F32)
        sent_in = nc.declare_dram_parameter("s_in", [8, 4], F32, isOutput=False)
        sent_out = nc.declare_dram_parameter("s_out", [8, 4], F32, isOutput=True)
    else:
        xr_h = nc.declare_dram_parameter("xr", [G, 64, MH], MM_DT, isOutput=False)
        w_h = nc.declare_dram_parameter("w", [128, G * 3 * 128], MM_DT, isOutput=False)
        y_h = nc.declare_dram_parameter("y", [G, 128, M], F32, isOutput=True)

    with TileContext(nc) as tc:
        with (
            tc.tile_pool(name="wpool", bufs=1) as wp,
            tc.tile_pool(name="xpool", bufs=3) as xp,
            tc.tile_pool(name="psum", bufs=6, space="PSUM") as pp,
            tc.tile_pool(name="psumd", bufs=1, space="PSUM") as pd,
            tc.tile_pool(name="ypool", bufs=64) as yp,
        ):
            wt = wp.tile([128, G * 3 * 128], MM_DT)
            nc.sync.dma_start(out=wt[:], in_=w_h[:])
            if internal_io:
                nc.sync.dma_start(out=sent_out[:], in_=sent_in[:])
            dummy = pd.tile([2, 2], F32)
            nc.tensor.matmul(dummy[:], wt[0:2, 0:2], wt[0:2, 0:2],
                             start=True, stop=True)
            for _ in range(loop_n or 1):
                ncopy = 0
                for g in range(G):
                    xg = xp.tile([128, MH], MM_DT)
                    nc.sync.dma_start(out=xg[0:64, :], in_=xr_h[g])
                    nc.gpsimd.tensor_copy(out=xg[64:128, 0:MH - 2],
                                          in_=xg[0:64, 2:MH])
                    for i in range(NTILES):
                        n0 = NT * i
                        ps = pp.tile([128, NT], F32)
                        wof = g * 3 * 128
                        nc.tensor.matmul(ps[:], wt[:, wof:wof + 128],
                                         xg[:, n0:n0 + NT],
                                         start=True, stop=False)
                        nc.tensor.matmul(ps[:], wt[:, wof + 128:wof + 256],
                                         xg[:, n0 + 1:n0 + 1 + NT],
                                         start=False, stop=False)
                        nc.tensor.matmul(ps[:], wt[0:64, wof + 256:wof + 384],
                                         xg[0:64, n0 + 4:n0 + 4 + NT],
                                         start=False, stop=True)
                        yt = yp.tile([128, NT], F32)
                        if ncopy % 2 == 0:
                            nc.vector.tensor_copy(out=yt[:], in_=ps[:])
                        else:
                            nc.scalar.copy(out=yt[:], in_=ps[:])
                        ncopy += 1
                        nc.sync.dma_start(out=y_h[g, :, n0:n0 + NT], in_=yt[:])
    nc.compile()
    return nc


def _host_prep_v3(x, params, rel_idx):
    x2 = np.ascontiguousarray(np.asarray(x, dtype=np.float32)[np.asarray(rel_idx).reshape(-1)])
    x_pad = np.pad(x2, ((0, 0), (PAD1, 17)))
    xr_cores = []
    for c in range(NCORES):
        xs = x_pad[:, c * LS: c * LS + LS + 32]
        arr = xs.reshape(G, J, MH, 8).transpose(0, 1, 3, 2)      # [g, j, r, m]
        xr_cores.append(arr.reshape(G, 64, MH).astype(_NP_IN_DT))

    p = np.asarray(params, dtype=np.float32)
    o_i = np.arange(8)
    r_i = np.arange(8)
    W5 = np.zeros((G, 5, 64, 128), dtype=np.float32)
    for g in range(G):
        for q in range(5):
            w_mat = 8 * q + r_i[:, None] - o_i[None, :]
            valid = (w_mat >= 0) & (w_mat <= 30)
            wm = np.where(valid, w_mat, 0)
            blk = p[g][:, :, wm] * valid[None, None]
            W5[g, q] = blk.transpose(1, 2, 0, 3).reshape(64, 128)
    # [128, G*3*128]: per group three lhsT mats A=[q0;q2], B=[q1;q3], C=[q4;0]
    wq3 = np.zeros((G, 3, 128, 128), dtype=np.float32)
    wq3[:, 0, 0:64], wq3[:, 0, 64:128] = W5[:, 0], W5[:, 2]
    wq3[:, 1, 0:64], wq3[:, 1, 64:128] = W5[:, 1], W5[:, 3]
    wq3[:, 2, 0:64] = W5[:, 4]
    wq3 = np.ascontiguousarray(
        wq3.transpose(2, 0, 1, 3).reshape(128, G * 3 * 128)).astype(_NP_IN_DT)
    return xr_cores, wq3, x2, p


def _host_prep(x, params, rel_idx):
    x2 = np.ascontiguousarray(np.asarray(x, dtype=np.float32)[np.asarray(rel_idx).reshape(-1)])
    x_pad = np.pad(x2, ((0, 0), (PAD1, 17)))                     # [64, L+32]
    xb_cores = []
    for c in range(NCORES):
        xs = x_pad[:, c * LS: c * LS + LS + 32]                  # [64, 8*MH]
        arr = xs.reshape(G, J, MH, 8).transpose(0, 1, 3, 2)      # [g, j, r, m]
        xb_cores.append(np.ascontiguousarray(
            arr.reshape(4, 128, MH)).astype(_NP_IN_DT))

    p = np.asarray(params, dtype=np.float32)
    wq = np.zeros((4, 5, 128, 128), dtype=np.float32)
    o_i = np.arange(8)
    r_i = np.arange(8)
    for g in range(G):
        t, half = g // 2, g % 2
        for q in range(5):
            w_mat = 8 * q + r_i[:, None] - o_i[None, :]          # [r, o]
            valid = (w_mat >= 0) & (w_mat <= 30)
            wm = np.where(valid, w_mat, 0)
            blk = p[g][:, :, wm] * valid[None, None]             # [f, j, r, o]
            blk = blk.transpose(1, 2, 0, 3).reshape(64, 128)     # [(j,r), (f,o)]
            wq[t, q, 64 * half:64 * half + 64, :] = blk
    # device layout: [128 partitions, (t, q, m) flattened]
    wq = np.ascontiguousarray(
        wq.transpose(2, 0, 1, 3).reshape(128, 4 * 5 * 128)).astype(_NP_IN_DT)
    return xb_cores, wq, x2, p


def _host_post(y_cores, x2, p):
    parts = [
        y.reshape(G, FG, 8, M).transpose(0, 1, 3, 2).reshape(G, FG, LS)
         .astype(np.float32)
        for y in y_cores
    ]
    y_full = np.concatenate(parts, axis=2)                       # [G, FG, L]

    xg = x2.reshape(G, J, L)
    pl = np.einsum("gjw,gfjw->gfw", xg[:, :, :W], p)
    left_c = np.cumsum(pl, axis=-1)
    y_full[:, :, :PAD1] = left_c[:, :, W - PAD1 - 1: W - 1]
    pr = np.einsum("gjw,gfjw->gfw", xg[:, :, L - W:], p)
    right_c = np.cumsum(pr[:, :, ::-1], axis=-1)[:, :, ::-1]
    n_right = W - 1 - PAD1
    y_full[:, :, L - n_right:] = right_c[:, :, 1: W - PAD1]
    return np.ascontiguousarray(y_full.reshape(F * L, 1), dtype=np.float32)


def _build_fn(nc):
    """Jitted 8-core shard_map executor for the compiled Bass module.
    Zero-init output buffers are created on device (no host upload)."""
    import jax
    import jax.numpy as jnp
    from jax.sharding import Mesh, PartitionSpec
    from jax.experimental.shard_map import shard_map
    from concourse.bass2jax import (
        _bass_exec_p, install_neuronx_cc_hook, partition_id_tensor)

    install_neuronx_cc_hook()
    partition_name = nc.partition_id_tensor.name if nc.partition_id_tensor else None
    in_names, out_names, out_avals = [], [], []
    for alloc in nc.m.functions[0].allocations:
        if not isinstance(alloc, mybir.MemoryLocationSet):
            continue
        name = alloc.memorylocations[0].name
        if alloc.kind == "ExternalInput":
            if name != partition_name:
                in_names.append(name)
        elif alloc.kind == "ExternalOutput":
            out_names.append(name)
            out_avals.append(jax.core.ShapedArray(
                tuple(alloc.tensor_shape), mybir.dt.np(alloc.dtype)))
    all_names = list(in_names) + list(out_names)
    if partition_name is not None:
        all_names.append(partition_name)

    def _body(*args):
        operands = list(args)
        if partition_name is not None:
            operands.append(partition_id_tensor())
        return tuple(_bass_exec_p.bind(
            *operands,
            out_avals=tuple(out_avals),
            in_names=tuple(all_names),
            out_names=tuple(out_names),
            lowering_input_output_aliases=(),
            sim_require_finite=True,
            sim_require_nnan=True,
            nc=nc,
        ))

    devices = jax.devices()[:NCORES]
    mesh = Mesh(np.asarray(devices), ("core",))
    nin = len(in_names) + len(out_avals)
    fn = jax.jit(shard_map(
        _body, mesh=mesh,
        in_specs=(PartitionSpec("core"),) * nin,
        out_specs=(PartitionSpec("core"),) * len(out_names),
        check_rep=False))
    # zero output buffers, materialized directly on device (no upload)
    sh = jax.sharding.NamedSharding(mesh, PartitionSpec("core"))
    zeros = [
        jax.jit(lambda av=av: jnp.zeros((NCORES * av.shape[0],) + av.shape[1:],
                                        av.dtype), out_shardings=sh)()
        for av in out_avals
    ]
    return fn, in_names, out_names, zeros


def kernel(x, params, rel_idx, _trace=False, _trace_out=None):
    if "nc" not in _cache:
        _cache["nc"] = _build_bass_v5()
        _cache["fn"] = _build_fn(_cache["nc"])
    nc = _cache["nc"]

    xr_cores, wq3, x2, p = _host_prep_v3(x, params, rel_idx)
    try:
        fn, in_names, out_names, zeros = _cache["fn"]
        per = {"xr": np.stack(xr_cores),
               "w": np.broadcast_to(wq3, (NCORES,) + wq3.shape)}
        concat = [np.ascontiguousarray(per[nm].reshape(
            NCORES * per[nm].shape[1], *per[nm].shape[2:])) for nm in in_names]
        outs = fn(*concat, *zeros)
        yi = out_names.index("y")
        y_all = np.asarray(outs[yi]).reshape(NCORES, G, 128, M)
        y_cores = [y_all[c] for c in range(NCORES)]
    except Exception:
        # fallback: reference SPMD runner
        in_maps = [{"xr": xr_cores[c], "w": wq3} for c in range(NCORES)]
        res = run_bass_kernel_spmd(nc, in_maps, list(range(NCORES)))
        y_cores = [np.asarray(res.results[c]["y"]) for c in range(NCORES)]
    return _host_post(y_cores, x2, p)


# revision 47
# speedup vs baseline: 10.2983x; 1.1121x over previous
"""Trainium2 Bass kernel for nn_DiagnoerMinBlcokScan (grouped 1D conv,
G=8 groups x FG=16 filters x J=8 channels, W=31 window, L=262144).

Strategy: data-parallel over L across 8 cores (no collectives; host slices
haloed shards). Inside each core the conv is phase-packed: output phases
o=0..7 fold into the matmul M dim (M = 16 filters x 8 phases = 128) and the
input is 8-phase deinterleaved so each of 5 "q" matmuls (contract = 64
(j,r) rows) reads a plain contiguous slice of SBUF. PSUM accumulates the 5
matmuls; result [128, n] tiles stream back to HBM. Group pairs share the
128-partition SBUF tiles (lower/upper 64 rows) so DMA runs full-width and
the PE can overlap row-disjoint matmuls.

Self-contained: hardcodes all shapes; host does the cheap boundary columns
(truncated-window semantics of the reference) and the phase re-interleave.
"""
import numpy as np

import concourse.bacc as bacc
import concourse.bass as bass
import concourse.mybir as mybir
from concourse.bass_utils import run_bass_kernel_spmd
from concourse.tile import TileContext
from concourse.tile_rust import add_dep_helper

D, L = 64, 262144
G, J, FG, W = 8, 8, 16, 31
PAD1 = 15
F = G * FG
NCORES = 8
LS = L // NCORES            # 32768 output cols per core
M = LS // 8                 # 4096 matmul free positions per core
MH = M + 4                  # deinterleaved cols incl. halo (n+q, q in 0..4)
NT = 512                    # matmul free-dim tile (one PSUM bank)
NTILES = M // NT            # 8

import os
# float16: full-rate PE streaming (1 col/cycle) with 10-bit mantissa;
# fp32 PSUM accumulation. Measured rel err ~3e-4 end to end.
_DT_NAME = os.environ.get("KERNEL_MM_DT", "float16")
MM_DT = getattr(mybir.dt, _DT_NAME)
F32 = mybir.dt.float32
# Output leaves the device as fp16 (halves the dominant DMA stream; host
# upcasts). Adds ~1.3e-4 L2 error on top of the fp16-compute 3e-4.
Y_DT = getattr(mybir.dt, os.environ.get("KERNEL_Y_DT", "float16"))
_NP_IN_DT = mybir.dt.np(MM_DT)
_NP_Y_DT = mybir.dt.np(Y_DT)

_cache = {}


def _build_bass(loop_n=None, internal_io=False):
    """loop_n: if set, emit the body N times (for slope timing).
    internal_io: DRAM traffic goes to internal scratch (timing-only builds,
    avoids shipping real data through the PJRT tunnel)."""
    nc = bacc.Bacc()
    if internal_io:
        xb_h = nc.dram_tensor("xb_i", [4, 128, MH], MM_DT)
        w_h = nc.dram_tensor("w_i", [128, 4 * 5 * 128], MM_DT)
        y_h = nc.dram_tensor("y_i", [G, 128, M], F32)
        sent_in = nc.declare_dram_parameter("s_in", [8, 4], F32, isOutput=False)
        sent_out = nc.declare_dram_parameter("s_out", [8, 4], F32, isOutput=True)
        nc._sentinel = (sent_in, sent_out)
    else:
        xb_h = nc.declare_dram_parameter("xb", [4, 128, MH], MM_DT, isOutput=False)
        w_h = nc.declare_dram_parameter("w", [128, 4 * 5 * 128], MM_DT, isOutput=False)
        y_h = nc.declare_dram_parameter("y", [G, 128, M], F32, isOutput=True)

    with TileContext(nc) as tc:
        with (
            tc.tile_pool(name="wpool", bufs=1) as wp,
            tc.tile_pool(name="xpool", bufs=2) as xp,
            tc.tile_pool(name="psum", bufs=5, space="PSUM") as pp,
            tc.tile_pool(name="psumb", bufs=1, space="PSUM") as pb,
            tc.tile_pool(name="psumd", bufs=1, space="PSUM") as pd,
            tc.tile_pool(name="ypool", bufs=64) as yp,
        ):
            # all weights resident: [128, 20*128] fp32 = 1.3 MB
            wt = wp.tile([128, 4 * 5 * 128], MM_DT)
            nc.sync.dma_start(out=wt[:], in_=w_h[:])
            if internal_io:
                si, so = nc._sentinel
                nc.sync.dma_start(out=so[:], in_=si[:])

            # Engine (TPB) instructions may carry at most ONE sync wait, so
            # the structure below keeps every matmul/copy at <=1:
            #  - a throwaway matmul reading only `wt` absorbs the weight-DMA
            #    wait into the PE's vector clock (data-dep waits propagate;
            #    explicit-dep nops do not),
            #  - each pair's first group uses a dedicated PSUM slot whose
            #    release is a pair-old DVE tick the PE has already observed,
            #    so its matmul only waits on the xt DMA,
            #  - y staging tiles are never reused (bufs=64) so copies only
            #    wait on the PE.
            dummy = pd.tile([2, 2], F32)
            dmm = nc.tensor.matmul(dummy[:], wt[0:2, 0:2], wt[0:2, 0:2],
                                   start=True, stop=True)

            # HW For_i loops crash the exec unit in this environment; for
            # slope timing we statically unroll the body instead.
            for _ in range(loop_n or 1):
                _emit_body(nc, tc, wp, xp, pp, pb, pd, yp, wt, dmm, xb_h, y_h)
    nc.compile()
    return nc


def _emit_body(nc, tc, wp, xp, pp, pb, pd, yp, wt, dmm, xb_h, y_h):
    if True:
        if True:
            ncopy = 0
            for t in range(4):
                xt = xp.tile([128, MH], MM_DT)
                nc.sync.dma_start(out=xt[:], in_=xb_h[t])
                for i in range(NTILES):
                    n0 = NT * i
                    for half in range(2):
                        g = 2 * t + half
                        sl = slice(64 * half, 64 * half + 64)
                        boundary = (i == 0 and half == 0)
                        ps = (pb if boundary else pp).tile([128, NT], F32)
                        for q in range(5):
                            mm = nc.tensor.matmul(
                                ps[:],
                                wt[sl, (t * 5 + q) * 128:(t * 5 + q) * 128 + 128],
                                xt[sl, n0 + q: n0 + q + NT],
                                start=(q == 0),
                                stop=(q == 4),
                            )
                            if t == 0 and boundary and q == 0:
                                add_dep_helper(mm.ins, dmm.ins, sync=False,
                                               reason="order after wt gate")
                        yt = yp.tile([128, NT], F32)
                        if boundary or ncopy % 2 == 0:
                            nc.vector.tensor_copy(out=yt[:], in_=ps[:])
                        else:
                            nc.scalar.copy(out=yt[:], in_=ps[:])
                        ncopy += 1
                        nc.sync.dma_start(out=y_h[g, :, n0:n0 + NT], in_=yt[:])


def _build_bass_v5(loop_n=None, internal_io=False):
    """Production kernel: 3-matmul decomposition. q-pairs (0,2) and (1,3)
    pack into contract-128 matmuls against a [xr ; xr shifted +2 cols] SBUF
    tile whose upper half is built on-chip by a DVE copy; q=4 rides alone at
    contract 64. All 8 group input DMAs + duplicates are hoisted to the
    front (x tiles fully resident) so input DMAs never queue behind output
    DMAs; weights stream in per-group chunks so the PE starts on group 0
    after ~100 KB of DMA. Cost model: 64.2 us/core vs 62.4 us DMA floor."""
    nc = bacc.Bacc()
    if internal_io:
        xr_h = nc.dram_tensor("xr_i", [G, 64, MH], MM_DT)
        w_h = nc.dram_tensor("w_i", [128, G * 3 * 128], MM_DT)
        y_h = nc.dram_tensor("y_i", [G, 128, M], Y_DT)
        sent_in = nc.declare_dram_parameter("s_in", [8, 4], F32, isOutput=False)
        sent_out = nc.declare_dram_parameter("s_out", [8, 4], F32, isOutput=True)
    else:
        xr_h = nc.declare_dram_parameter("xr", [G, 64, MH], MM_DT, isOutput=False)
        w_h = nc.declare_dram_parameter("w", [128, G * 3 * 128], MM_DT, isOutput=False)
        y_h = nc.declare_dram_parameter("y", [G, 128, M], Y_DT, isOutput=True)

    with TileContext(nc) as tc:
        with (
            tc.tile_pool(name="wpool", bufs=1) as wp,
            tc.tile_pool(name="xpool", bufs=8) as xp,
            tc.tile_pool(name="psum", bufs=6, space="PSUM") as pp,
            tc.tile_pool(name="psumd", bufs=1, space="PSUM") as pd,
            tc.tile_pool(name="ypool", bufs=16) as yp,
        ):
            wt = wp.tile([128, G * 3 * 128], MM_DT)
            xgs = []
            # group 0 streams in two chunks so the PE starts after ~130 KB:
            # chunk A covers output tiles 0-1 (cols < 1032), chunk B the rest.
            CA = 1032
            xg0 = xp.tile([128, MH], MM_DT)
            nc.sync.dma_start(out=xg0[0:64, 0:CA], in_=xr_h[0][:, 0:CA])
            nc.sync.dma_start(out=wt[:, 0:384], in_=w_h[:, 0:384])
            nc.vector.tensor_copy(out=xg0[64:128, 0:CA - 2], in_=xg0[0:64, 2:CA])
            nc.sync.dma_start(out=xg0[0:64, CA:MH], in_=xr_h[0][:, CA:MH])
            nc.vector.tensor_copy(out=xg0[64:128, CA - 2:MH - 2],
                                  in_=xg0[0:64, CA:MH])
            dummy = pd.tile([2, 2], F32)
            nc.tensor.matmul(dummy[:], wt[0:2, 0:2], wt[0:2, 0:2],
                             start=True, stop=True)
            xgs.append(xg0)
            if internal_io:
                nc.sync.dma_start(out=sent_out[:], in_=sent_in[:])
            for g in range(1, G):
                xg = xp.tile([128, MH], MM_DT)
                nc.sync.dma_start(out=xg[0:64, :], in_=xr_h[g])
                nc.sync.dma_start(out=wt[:, g * 384:(g + 1) * 384],
                                  in_=w_h[:, g * 384:(g + 1) * 384])
                xgs.append(xg)
            for g in range(1, G):
                nc.vector.tensor_copy(out=xgs[g][64:128, 0:MH - 2],
                                      in_=xgs[g][0:64, 2:MH])
            for _ in range(loop_n or 1):
                ncopy = 0
                for g in range(G):
                    xg = xgs[g]
                    # batch 4 output tiles per DMA (HWDGE enqueues are the
                    # serial resource); taper the last group so the final
                    # copy->DMA chain is short.
                    batches = [4, 4] if g < G - 1 else [4, 2, 1, 1]
                    i = 0
                    for bsz in batches:
                        yt = yp.tile([128, bsz * NT], Y_DT)
                        b0 = NT * i
                        for j in range(bsz):
                            n0 = NT * i
                            ps = pp.tile([128, NT], F32)
                            wof = g * 3 * 128
                            nc.tensor.matmul(ps[:], wt[:, wof:wof + 128],
                                             xg[:, n0:n0 + NT],
                                             start=True, stop=False)
                            nc.tensor.matmul(ps[:], wt[:, wof + 128:wof + 256],
                                             xg[:, n0 + 1:n0 + 1 + NT],
                                             start=False, stop=False)
                            nc.tensor.matmul(ps[:], wt[0:64, wof + 256:wof + 384],
                                             xg[0:64, n0 + 4:n0 + 4 + NT],
                                             start=False, stop=True)
                            dst = yt[:, j * NT:(j + 1) * NT]
                            if ncopy % 2 == 0:
                                nc.vector.tensor_copy(out=dst, in_=ps[:])
                            else:
                                nc.scalar.copy(out=dst, in_=ps[:])
                            ncopy += 1
                            i += 1
                        nc.sync.dma_start(out=y_h[g, :, b0:b0 + bsz * NT],
                                          in_=yt[:])
    nc.compile()
    return nc


def _build_bass_v3(loop_n=None, internal_io=False):
    """3-matmul variant: q-pairs (0,2) and (1,3) packed into contract-128
    matmuls against a [xr ; xr shifted +2] SBUF tile built on-chip by
    gpsimd; q=4 rides alone at contract 64. PE streams 3x512 cols per
    group-tile instead of 5x512."""
    nc = bacc.Bacc()
    if internal_io:
        xr_h = nc.dram_tensor("xr_i", [G, 64, MH], MM_DT)
        w_h = nc.dram_tensor("w_i", [128, G * 3 * 128], MM_DT)
        y_h = nc.dram_tensor("y_i", [G, 128, M], F32)
        sent_in = nc.declare_dram_parameter("s_in", [8, 4], F32, isOutput=False)
        sent_out = nc.declare_dram_parameter("s_out", [8, 4], F32, isOutput=True)
    else:
        xr_h = nc.declare_dram_parameter("xr", [G, 64, MH], MM_DT, isOutput=False)
        w_h = nc.declare_dram_parameter("w", [128, G * 3 * 128], MM_DT, isOutput=False)
        y_h = nc.declare_dram_parameter("y", [G, 128, M], F32, isOutput=True)

    with TileContext(nc) as tc:
        with (
            tc.tile_pool(name="wpool", bufs=1) as wp,
            tc.tile_pool(name="xpool", bufs=3) as xp,
            tc.tile_pool(name="psum", bufs=6, space="PSUM") as pp,
            tc.tile_pool(name="psumd", bufs=1, space="PSUM") as pd,
            tc.tile_pool(name="ypool", bufs=64) as yp,
        ):
            wt = wp.tile([128, G * 3 * 128], MM_DT)
            nc.sync.dma_start(out=wt[:], in_=w_h[:])
            if internal_io:
                nc.sync.dma_start(out=sent_out[:], in_=sent_in[:])
            dummy = pd.tile([2, 2], F32)
            nc.tensor.matmul(dummy[:], wt[0:2, 0:2], wt[0:2, 0:2],
                             start=True, stop=True)
            for _ in range(loop_n or 1):
                ncopy = 0
                for g in range(G):
                    xg = xp.tile([128, MH], MM_DT)
                    nc.sync.dma_start(out=xg[0:64, :], in_=xr_h[g])
                    nc.gpsimd.tensor_copy(out=xg[64:128, 0:MH - 2],
                                          in_=xg[0:64, 2:MH])
                    for i in range(NTILES):
                        n0 = NT * i
                        ps = pp.tile([128, NT], F32)
                        wof = g * 3 * 128
                        nc.tensor.matmul(ps[:], wt[:, wof:wof + 128],
                                         xg[:, n0:n0 + NT],
                                         start=True, stop=False)
                        nc.tensor.matmul(ps[:], wt[:, wof + 128:wof + 256],
                                         xg[:, n0 + 1:n0 + 1 + NT],
                                         start=False, stop=False)
                        nc.tensor.matmul(ps[:], wt[0:64, wof + 256:wof + 384],
                                         xg[0:64, n0 + 4:n0 + 4 + NT],
                                         start=False, stop=True)
                        yt = yp.tile([128, NT], F32)
                        if ncopy % 2 == 0:
                            nc.vector.tensor_copy(out=yt[:], in_=ps[:])
                        else:
                            nc.scalar.copy(out=yt[:], in_=ps[:])
                        ncopy += 1
                        nc.sync.dma_start(out=y_h[g, :, n0:n0 + NT], in_=yt[:])
    nc.compile()
    return nc


def _host_prep_v3(x, params, rel_idx):
    x2 = np.ascontiguousarray(np.asarray(x, dtype=np.float32)[np.asarray(rel_idx).reshape(-1)])
    x_pad = np.pad(x2, ((0, 0), (PAD1, 17)))
    xr_cores = []
    for c in range(NCORES):
        xs = x_pad[:, c * LS: c * LS + LS + 32]
        arr = xs.reshape(G, J, MH, 8).transpose(0, 1, 3, 2)      # [g, j, r, m]
        xr_cores.append(arr.reshape(G, 64, MH).astype(_NP_IN_DT))

    p = np.asarray(params, dtype=np.float32)
    o_i = np.arange(8)
    r_i = np.arange(8)
    W5 = np.zeros((G, 5, 64, 128), dtype=np.float32)
    for g in range(G):
        for q in range(5):
            w_mat = 8 * q + r_i[:, None] - o_i[None, :]
            valid = (w_mat >= 0) & (w_mat <= 30)
            wm = np.where(valid, w_mat, 0)
            blk = p[g][:, :, wm] * valid[None, None]
            W5[g, q] = blk.transpose(1, 2, 0, 3).reshape(64, 128)
    # [128, G*3*128]: per group three lhsT mats A=[q0;q2], B=[q1;q3], C=[q4;0]
    wq3 = np.zeros((G, 3, 128, 128), dtype=np.float32)
    wq3[:, 0, 0:64], wq3[:, 0, 64:128] = W5[:, 0], W5[:, 2]
    wq3[:, 1, 0:64], wq3[:, 1, 64:128] = W5[:, 1], W5[:, 3]
    wq3[:, 2, 0:64] = W5[:, 4]
    wq3 = np.ascontiguousarray(
        wq3.transpose(2, 0, 1, 3).reshape(128, G * 3 * 128)).astype(_NP_IN_DT)
    return xr_cores, wq3, x2, p


def _host_prep(x, params, rel_idx):
    x2 = np.ascontiguousarray(np.asarray(x, dtype=np.float32)[np.asarray(rel_idx).reshape(-1)])
    x_pad = np.pad(x2, ((0, 0), (PAD1, 17)))                     # [64, L+32]
    xb_cores = []
    for c in range(NCORES):
        xs = x_pad[:, c * LS: c * LS + LS + 32]                  # [64, 8*MH]
        arr = xs.reshape(G, J, MH, 8).transpose(0, 1, 3, 2)      # [g, j, r, m]
        xb_cores.append(np.ascontiguousarray(
            arr.reshape(4, 128, MH)).astype(_NP_IN_DT))

    p = np.asarray(params, dtype=np.float32)
    wq = np.zeros((4, 5, 128, 128), dtype=np.float32)
    o_i = np.arange(8)
    r_i = np.arange(8)
    for g in range(G):
        t, half = g // 2, g % 2
        for q in range(5):
            w_mat = 8 * q + r_i[:, None] - o_i[None, :]          # [r, o]
            valid = (w_mat >= 0) & (w_mat <= 30)
            wm = np.where(valid, w_mat, 0)
            blk = p[g][:, :, wm] * valid[None, None]             # [f, j, r, o]
            blk = blk.transpose(1, 2, 0, 3).reshape(64, 128)     # [(j,r), (f,o)]
            wq[t, q, 64 * half:64 * half + 64, :] = blk
    # device layout: [128 partitions, (t, q, m) flattened]
    wq = np.ascontiguousarray(
        wq.transpose(2, 0, 1, 3).reshape(128, 4 * 5 * 128)).astype(_NP_IN_DT)
    return xb_cores, wq, x2, p


def _host_post(y_cores, x2, p):
    parts = [
        y.reshape(G, FG, 8, M).transpose(0, 1, 3, 2).reshape(G, FG, LS)
         .astype(np.float32)
        for y in y_cores
    ]
    y_full = np.concatenate(parts, axis=2)                       # [G, FG, L]

    xg = x2.reshape(G, J, L)
    pl = np.einsum("gjw,gfjw->gfw", xg[:, :, :W], p)
    left_c = np.cumsum(pl, axis=-1)
    y_full[:, :, :PAD1] = left_c[:, :, W - PAD1 - 1: W - 1]
    pr = np.einsum("gjw,gfjw->gfw", xg[:, :, L - W:], p)
    right_c = np.cumsum(pr[:, :, ::-1], axis=-1)[:, :, ::-1]
    n_right = W - 1 - PAD1
    y_full[:, :, L - n_right:] = right_c[:, :, 1: W - PAD1]
    return np.ascontiguousarray(y_full.reshape(F * L, 1), dtype=np.float32)


def _build_fn(nc):
    """Jitted 8-core shard_map executor for the compiled Bass module.
    Zero-init output buffers are created on device (no host upload)."""
    import jax
    import jax.numpy as jnp
    from jax.sharding import Mesh, PartitionSpec
    from jax.experimental.shard_map import shard_map
    from concourse.bass2jax import (
        _bass_exec_p, install_neuronx_cc_hook, partition_id_tensor)

    install_neuronx_cc_hook()
    partition_name = nc.partition_id_tensor.name if nc.partition_id_tensor else None
    in_names, out_names, out_avals = [], [], []
    for alloc in nc.m.functions[0].allocations:
        if not isinstance(alloc, mybir.MemoryLocationSet):
            continue
        name = alloc.memorylocations[0].name
        if alloc.kind == "ExternalInput":
            if name != partition_name:
                in_names.append(name)
        elif alloc.kind == "ExternalOutput":
            out_names.append(name)
            out_avals.append(jax.core.ShapedArray(
                tuple(alloc.tensor_shape), mybir.dt.np(alloc.dtype)))
    all_names = list(in_names) + list(out_names)
    if partition_name is not None:
        all_names.append(partition_name)

    def _body(*args):
        operands = list(args)
        if partition_name is not None:
            operands.append(partition_id_tensor())
        return tuple(_bass_exec_p.bind(
            *operands,
            out_avals=tuple(out_avals),
            in_names=tuple(all_names),
            out_names=tuple(out_names),
            lowering_input_output_aliases=(),
            sim_require_finite=True,
            sim_require_nnan=True,
            nc=nc,
        ))

    devices = jax.devices()[:NCORES]
    mesh = Mesh(np.asarray(devices), ("core",))
    nin = len(in_names) + len(out_avals)
    fn = jax.jit(shard_map(
        _body, mesh=mesh,
        in_specs=(PartitionSpec("core"),) * nin,
        out_specs=(PartitionSpec("core"),) * len(out_names),
        check_rep=False))
    # zero output buffers, materialized directly on device (no upload)
    sh = jax.sharding.NamedSharding(mesh, PartitionSpec("core"))
    zeros = [
        jax.jit(lambda av=av: jnp.zeros((NCORES * av.shape[0],) + av.shape[1:],
                                        av.dtype), out_shardings=sh)()
        for av in out_avals
    ]
    return fn, in_names, out_names, zeros


def kernel(x, params, rel_idx, _trace=False, _trace_out=None):
    if "nc" not in _cache:
        _cache["nc"] = _build_bass_v5()
        _cache["fn"] = _build_fn(_cache["nc"])
    nc = _cache["nc"]

    xr_cores, wq3, x2, p = _host_prep_v3(x, params, rel_idx)
    try:
        fn, in_names, out_names, zeros = _cache["fn"]
        per = {"xr": np.stack(xr_cores),
               "w": np.broadcast_to(wq3, (NCORES,) + wq3.shape)}
        concat = [np.ascontiguousarray(per[nm].reshape(
            NCORES * per[nm].shape[1], *per[nm].shape[2:])) for nm in in_names]
        outs = fn(*concat, *zeros)
        yi = out_names.index("y")
        y_all = np.asarray(outs[yi]).reshape(NCORES, G, 128, M)
        y_cores = [y_all[c] for c in range(NCORES)]
    except Exception:
        # fallback: reference SPMD runner
        in_maps = [{"xr": xr_cores[c], "w": wq3} for c in range(NCORES)]
        res = run_bass_kernel_spmd(nc, in_maps, list(range(NCORES)))
        y_cores = [np.asarray(res.results[c]["y"]) for c in range(NCORES)]
    return _host_post(y_cores, x2, p)


# revision 48
# speedup vs baseline: 10.5111x; 1.0207x over previous
"""Trainium2 Bass kernel for nn_DiagnoerMinBlcokScan (grouped 1D conv,
G=8 groups x FG=16 filters x J=8 channels, W=31 window, L=262144).

Strategy: data-parallel over L across 8 cores (no collectives; host slices
haloed shards). Inside each core the conv is phase-packed: output phases
o=0..7 fold into the matmul M dim (M = 16 filters x 8 phases = 128) and the
input is 8-phase deinterleaved so each of 5 "q" matmuls (contract = 64
(j,r) rows) reads a plain contiguous slice of SBUF. PSUM accumulates the 5
matmuls; result [128, n] tiles stream back to HBM. Group pairs share the
128-partition SBUF tiles (lower/upper 64 rows) so DMA runs full-width and
the PE can overlap row-disjoint matmuls.

Self-contained: hardcodes all shapes; host does the cheap boundary columns
(truncated-window semantics of the reference) and the phase re-interleave.
"""
import numpy as np

import concourse.bacc as bacc
import concourse.bass as bass
import concourse.mybir as mybir
from concourse.bass_utils import run_bass_kernel_spmd
from concourse.tile import TileContext
from concourse.tile_rust import add_dep_helper

D, L = 64, 262144
G, J, FG, W = 8, 8, 16, 31
PAD1 = 15
F = G * FG
NCORES = 8
LS = L // NCORES            # 32768 output cols per core
M = LS // 8                 # 4096 matmul free positions per core
MH = M + 4                  # deinterleaved cols incl. halo (n+q, q in 0..4)
NT = 512                    # matmul free-dim tile (one PSUM bank)
NTILES = M // NT            # 8

import os
# float16: full-rate PE streaming (1 col/cycle) with 10-bit mantissa;
# fp32 PSUM accumulation. Measured rel err ~3e-4 end to end.
_DT_NAME = os.environ.get("KERNEL_MM_DT", "float16")
MM_DT = getattr(mybir.dt, _DT_NAME)
F32 = mybir.dt.float32
# Output leaves the device as fp16 (halves the dominant DMA stream; host
# upcasts). Adds ~1.3e-4 L2 error on top of the fp16-compute 3e-4.
Y_DT = getattr(mybir.dt, os.environ.get("KERNEL_Y_DT", "float16"))
_NP_IN_DT = mybir.dt.np(MM_DT)
_NP_Y_DT = mybir.dt.np(Y_DT)

_cache = {}


def _build_bass(loop_n=None, internal_io=False):
    """loop_n: if set, emit the body N times (for slope timing).
    internal_io: DRAM traffic goes to internal scratch (timing-only builds,
    avoids shipping real data through the PJRT tunnel)."""
    nc = bacc.Bacc()
    if internal_io:
        xb_h = nc.dram_tensor("xb_i", [4, 128, MH], MM_DT)
        w_h = nc.dram_tensor("w_i", [128, 4 * 5 * 128], MM_DT)
        y_h = nc.dram_tensor("y_i", [G, 128, M], F32)
        sent_in = nc.declare_dram_parameter("s_in", [8, 4], F32, isOutput=False)
        sent_out = nc.declare_dram_parameter("s_out", [8, 4], F32, isOutput=True)
        nc._sentinel = (sent_in, sent_out)
    else:
        xb_h = nc.declare_dram_parameter("xb", [4, 128, MH], MM_DT, isOutput=False)
        w_h = nc.declare_dram_parameter("w", [128, 4 * 5 * 128], MM_DT, isOutput=False)
        y_h = nc.declare_dram_parameter("y", [G, 128, M], F32, isOutput=True)

    with TileContext(nc) as tc:
        with (
            tc.tile_pool(name="wpool", bufs=1) as wp,
            tc.tile_pool(name="xpool", bufs=2) as xp,
            tc.tile_pool(name="psum", bufs=5, space="PSUM") as pp,
            tc.tile_pool(name="psumb", bufs=1, space="PSUM") as pb,
            tc.tile_pool(name="psumd", bufs=1, space="PSUM") as pd,
            tc.tile_pool(name="ypool", bufs=64) as yp,
        ):
            # all weights resident: [128, 20*128] fp32 = 1.3 MB
            wt = wp.tile([128, 4 * 5 * 128], MM_DT)
            nc.sync.dma_start(out=wt[:], in_=w_h[:])
            if internal_io:
                si, so = nc._sentinel
                nc.sync.dma_start(out=so[:], in_=si[:])

            # Engine (TPB) instructions may carry at most ONE sync wait, so
            # the structure below keeps every matmul/copy at <=1:
            #  - a throwaway matmul reading only `wt` absorbs the weight-DMA
            #    wait into the PE's vector clock (data-dep waits propagate;
            #    explicit-dep nops do not),
            #  - each pair's first group uses a dedicated PSUM slot whose
            #    release is a pair-old DVE tick the PE has already observed,
            #    so its matmul only waits on the xt DMA,
            #  - y staging tiles are never reused (bufs=64) so copies only
            #    wait on the PE.
            dummy = pd.tile([2, 2], F32)
            dmm = nc.tensor.matmul(dummy[:], wt[0:2, 0:2], wt[0:2, 0:2],
                                   start=True, stop=True)

            # HW For_i loops crash the exec unit in this environment; for
            # slope timing we statically unroll the body instead.
            for _ in range(loop_n or 1):
                _emit_body(nc, tc, wp, xp, pp, pb, pd, yp, wt, dmm, xb_h, y_h)
    nc.compile()
    return nc


def _emit_body(nc, tc, wp, xp, pp, pb, pd, yp, wt, dmm, xb_h, y_h):
    if True:
        if True:
            ncopy = 0
            for t in range(4):
                xt = xp.tile([128, MH], MM_DT)
                nc.sync.dma_start(out=xt[:], in_=xb_h[t])
                for i in range(NTILES):
                    n0 = NT * i
                    for half in range(2):
                        g = 2 * t + half
                        sl = slice(64 * half, 64 * half + 64)
                        boundary = (i == 0 and half == 0)
                        ps = (pb if boundary else pp).tile([128, NT], F32)
                        for q in range(5):
                            mm = nc.tensor.matmul(
                                ps[:],
                                wt[sl, (t * 5 + q) * 128:(t * 5 + q) * 128 + 128],
                                xt[sl, n0 + q: n0 + q + NT],
                                start=(q == 0),
                                stop=(q == 4),
                            )
                            if t == 0 and boundary and q == 0:
                                add_dep_helper(mm.ins, dmm.ins, sync=False,
                                               reason="order after wt gate")
                        yt = yp.tile([128, NT], F32)
                        if boundary or ncopy % 2 == 0:
                            nc.vector.tensor_copy(out=yt[:], in_=ps[:])
                        else:
                            nc.scalar.copy(out=yt[:], in_=ps[:])
                        ncopy += 1
                        nc.sync.dma_start(out=y_h[g, :, n0:n0 + NT], in_=yt[:])


def _build_bass_v5(loop_n=None, internal_io=False):
    """Production kernel: 3-matmul decomposition. q-pairs (0,2) and (1,3)
    pack into contract-128 matmuls against a [xr ; xr shifted +2 cols] SBUF
    tile whose upper half is built on-chip by a DVE copy; q=4 rides alone at
    contract 64. All 8 group input DMAs + duplicates are hoisted to the
    front (x tiles fully resident) so input DMAs never queue behind output
    DMAs; weights stream in per-group chunks so the PE starts on group 0
    after ~100 KB of DMA. Cost model: 64.2 us/core vs 62.4 us DMA floor."""
    nc = bacc.Bacc()
    if internal_io:
        xr_h = nc.dram_tensor("xr_i", [G, 64, MH], MM_DT)
        w_h = nc.dram_tensor("w_i", [128, G * 3 * 128], MM_DT)
        y_h = nc.dram_tensor("y_i", [G, 128, M], Y_DT)
        sent_in = nc.declare_dram_parameter("s_in", [8, 4], F32, isOutput=False)
        sent_out = nc.declare_dram_parameter("s_out", [8, 4], F32, isOutput=True)
    else:
        xr_h = nc.declare_dram_parameter("xr", [G, 64, MH], MM_DT, isOutput=False)
        w_h = nc.declare_dram_parameter("w", [128, G * 3 * 128], MM_DT, isOutput=False)
        y_h = nc.declare_dram_parameter("y", [G, 128, M], Y_DT, isOutput=True)

    with TileContext(nc) as tc:
        with (
            tc.tile_pool(name="wpool", bufs=1) as wp,
            tc.tile_pool(name="xpool", bufs=8) as xp,
            tc.tile_pool(name="psum", bufs=6, space="PSUM") as pp,
            tc.tile_pool(name="psumd", bufs=1, space="PSUM") as pd,
            tc.tile_pool(name="ypool", bufs=16) as yp,
        ):
            wt = wp.tile([128, G * 3 * 128], MM_DT)
            xgs = []
            # group 0 streams in a 3-chunk cascade (tile 0, tile 1, rest) so
            # the PE starts after ~65 KB of input and never waits again.
            xg0 = xp.tile([128, MH], MM_DT)
            chunks = [(0, 520), (520, 1032), (1032, MH)]
            a0, b0_ = chunks[0]
            nc.sync.dma_start(out=xg0[0:64, a0:b0_], in_=xr_h[0][:, a0:b0_])
            nc.sync.dma_start(out=wt[:, 0:384], in_=w_h[:, 0:384])
            nc.vector.tensor_copy(out=xg0[64:128, 0:b0_ - 2], in_=xg0[0:64, 2:b0_])
            for a, b in chunks[1:]:
                nc.sync.dma_start(out=xg0[0:64, a:b], in_=xr_h[0][:, a:b])
                nc.vector.tensor_copy(out=xg0[64:128, a - 2:b - 2],
                                      in_=xg0[0:64, a:b])
            dummy = pd.tile([2, 2], F32)
            nc.tensor.matmul(dummy[:], wt[0:2, 0:2], wt[0:2, 0:2],
                             start=True, stop=True)
            xgs.append(xg0)
            if internal_io:
                nc.sync.dma_start(out=sent_out[:], in_=sent_in[:])
            for g in range(1, G):
                xg = xp.tile([128, MH], MM_DT)
                nc.sync.dma_start(out=xg[0:64, :], in_=xr_h[g])
                nc.sync.dma_start(out=wt[:, g * 384:(g + 1) * 384],
                                  in_=w_h[:, g * 384:(g + 1) * 384])
                xgs.append(xg)
            for g in range(1, G):
                nc.vector.tensor_copy(out=xgs[g][64:128, 0:MH - 2],
                                      in_=xgs[g][0:64, 2:MH])
            for _ in range(loop_n or 1):
                ncopy = 0
                for g in range(G):
                    xg = xgs[g]
                    # batch 4 output tiles per DMA (HWDGE enqueues are the
                    # serial resource); taper the last group so the final
                    # copy->DMA chain is short.
                    batches = [4, 4] if g < G - 1 else [4, 2, 1, 1]
                    i = 0
                    for bsz in batches:
                        yt = yp.tile([128, bsz * NT], Y_DT)
                        b0 = NT * i
                        for j in range(bsz):
                            n0 = NT * i
                            ps = pp.tile([128, NT], F32)
                            wof = g * 3 * 128
                            nc.tensor.matmul(ps[:], wt[:, wof:wof + 128],
                                             xg[:, n0:n0 + NT],
                                             start=True, stop=False)
                            nc.tensor.matmul(ps[:], wt[:, wof + 128:wof + 256],
                                             xg[:, n0 + 1:n0 + 1 + NT],
                                             start=False, stop=False)
                            nc.tensor.matmul(ps[:], wt[0:64, wof + 256:wof + 384],
                                             xg[0:64, n0 + 4:n0 + 4 + NT],
                                             start=False, stop=True)
                            dst = yt[:, j * NT:(j + 1) * NT]
                            if ncopy % 2 == 0:
                                nc.vector.tensor_copy(out=dst, in_=ps[:])
                            else:
                                nc.scalar.copy(out=dst, in_=ps[:])
                            ncopy += 1
                            i += 1
                        nc.sync.dma_start(out=y_h[g, :, b0:b0 + bsz * NT],
                                          in_=yt[:])
    nc.compile()
    return nc


def _build_bass_v3(loop_n=None, internal_io=False):
    """3-matmul variant: q-pairs (0,2) and (1,3) packed into contract-128
    matmuls against a [xr ; xr shifted +2] SBUF tile built on-chip by
    gpsimd; q=4 rides alone at contract 64. PE streams 3x512 cols per
    group-tile instead of 5x512."""
    nc = bacc.Bacc()
    if internal_io:
        xr_h = nc.dram_tensor("xr_i", [G, 64, MH], MM_DT)
        w_h = nc.dram_tensor("w_i", [128, G * 3 * 128], MM_DT)
        y_h = nc.dram_tensor("y_i", [G, 128, M], F32)
        sent_in = nc.declare_dram_parameter("s_in", [8, 4], F32, isOutput=False)
        sent_out = nc.declare_dram_parameter("s_out", [8, 4], F32, isOutput=True)
    else:
        xr_h = nc.declare_dram_parameter("xr", [G, 64, MH], MM_DT, isOutput=False)
        w_h = nc.declare_dram_parameter("w", [128, G * 3 * 128], MM_DT, isOutput=False)
        y_h = nc.declare_dram_parameter("y", [G, 128, M], F32, isOutput=True)

    with TileContext(nc) as tc:
        with (
            tc.tile_pool(name="wpool", bufs=1) as wp,
            tc.tile_pool(name="xpool", bufs=3) as xp,
            tc.tile_pool(name="psum", bufs=6, space="PSUM") as pp,
            tc.tile_pool(name="psumd", bufs=1, space="PSUM") as pd,
            tc.tile_pool(name="ypool", bufs=64) as yp,
        ):
            wt = wp.tile([128, G * 3 * 128], MM_DT)
            nc.sync.dma_start(out=wt[:], in_=w_h[:])
            if internal_io:
                nc.sync.dma_start(out=sent_out[:], in_=sent_in[:])
            dummy = pd.tile([2, 2], F32)
            nc.tensor.matmul(dummy[:], wt[0:2, 0:2], wt[0:2, 0:2],
                             start=True, stop=True)
            for _ in range(loop_n or 1):
                ncopy = 0
                for g in range(G):
                    xg = xp.tile([128, MH], MM_DT)
                    nc.sync.dma_start(out=xg[0:64, :], in_=xr_h[g])
                    nc.gpsimd.tensor_copy(out=xg[64:128, 0:MH - 2],
                                          in_=xg[0:64, 2:MH])
                    for i in range(NTILES):
                        n0 = NT * i
                        ps = pp.tile([128, NT], F32)
                        wof = g * 3 * 128
                        nc.tensor.matmul(ps[:], wt[:, wof:wof + 128],
                                         xg[:, n0:n0 + NT],
                                         start=True, stop=False)
                        nc.tensor.matmul(ps[:], wt[:, wof + 128:wof + 256],
                                         xg[:, n0 + 1:n0 + 1 + NT],
                                         start=False, stop=False)
                        nc.tensor.matmul(ps[:], wt[0:64, wof + 256:wof + 384],
                                         xg[0:64, n0 + 4:n0 + 4 + NT],
                                         start=False, stop=True)
                        yt = yp.tile([128, NT], F32)
                        if ncopy % 2 == 0:
                            nc.vector.tensor_copy(out=yt[:], in_=ps[:])
                        else:
                            nc.scalar.copy(out=yt[:], in_=ps[:])
                        ncopy += 1
                        nc.sync.dma_start(out=y_h[g, :, n0:n0 + NT], in_=yt[:])
    nc.compile()
    return nc


def _host_prep_v3(x, params, rel_idx):
    x2 = np.ascontiguousarray(np.asarray(x, dtype=np.float32)[np.asarray(rel_idx).reshape(-1)])
    x_pad = np.pad(x2, ((0, 0), (PAD1, 17)))
    xr_cores = []
    for c in range(NCORES):
        xs = x_pad[:, c * LS: c * LS + LS + 32]
        arr = xs.reshape(G, J, MH, 8).transpose(0, 1, 3, 2)      # [g, j, r, m]
        xr_cores.append(arr.reshape(G, 64, MH).astype(_NP_IN_DT))

    p = np.asarray(params, dtype=np.float32)
    o_i = np.arange(8)
    r_i = np.arange(8)
    W5 = np.zeros((G, 5, 64, 128), dtype=np.float32)
    for g in range(G):
        for q in range(5):
            w_mat = 8 * q + r_i[:, None] - o_i[None, :]
            valid = (w_mat >= 0) & (w_mat <= 30)
            wm = np.where(valid, w_mat, 0)
            blk = p[g][:, :, wm] * valid[None, None]
            W5[g, q] = blk.transpose(1, 2, 0, 3).reshape(64, 128)
    # [128, G*3*128]: per group three lhsT mats A=[q0;q2], B=[q1;q3], C=[q4;0]
    wq3 = np.zeros((G, 3, 128, 128), dtype=np.float32)
    wq3[:, 0, 0:64], wq3[:, 0, 64:128] = W5[:, 0], W5[:, 2]
    wq3[:, 1, 0:64], wq3[:, 1, 64:128] = W5[:, 1], W5[:, 3]
    wq3[:, 2, 0:64] = W5[:, 4]
    wq3 = np.ascontiguousarray(
        wq3.transpose(2, 0, 1, 3).reshape(128, G * 3 * 128)).astype(_NP_IN_DT)
    return xr_cores, wq3, x2, p


def _host_prep(x, params, rel_idx):
    x2 = np.ascontiguousarray(np.asarray(x, dtype=np.float32)[np.asarray(rel_idx).reshape(-1)])
    x_pad = np.pad(x2, ((0, 0), (PAD1, 17)))                     # [64, L+32]
    xb_cores = []
    for c in range(NCORES):
        xs = x_pad[:, c * LS: c * LS + LS + 32]                  # [64, 8*MH]
        arr = xs.reshape(G, J, MH, 8).transpose(0, 1, 3, 2)      # [g, j, r, m]
        xb_cores.append(np.ascontiguousarray(
            arr.reshape(4, 128, MH)).astype(_NP_IN_DT))

    p = np.asarray(params, dtype=np.float32)
    wq = np.zeros((4, 5, 128, 128), dtype=np.float32)
    o_i = np.arange(8)
    r_i = np.arange(8)
    for g in range(G):
        t, half = g // 2, g % 2
        for q in range(5):
            w_mat = 8 * q + r_i[:, None] - o_i[None, :]          # [r, o]
            valid = (w_mat >= 0) & (w_mat <= 30)
            wm = np.where(valid, w_mat, 0)
            blk = p[g][:, :, wm] * valid[None, None]             # [f, j, r, o]
            blk = blk.transpose(1, 2, 0, 3).reshape(64, 128)     # [(j,r), (f,o)]
            wq[t, q, 64 * half:64 * half + 64, :] = blk
    # device layout: [128 partitions, (t, q, m) flattened]
    wq = np.ascontiguousarray(
        wq.transpose(2, 0, 1, 3).reshape(128, 4 * 5 * 128)).astype(_NP_IN_DT)
    return xb_cores, wq, x2, p


def _host_post(y_cores, x2, p):
    parts = [
        y.reshape(G, FG, 8, M).transpose(0, 1, 3, 2).reshape(G, FG, LS)
         .astype(np.float32)
        for y in y_cores
    ]
    y_full = np.concatenate(parts, axis=2)                       # [G, FG, L]

    xg = x2.reshape(G, J, L)
    pl = np.einsum("gjw,gfjw->gfw", xg[:, :, :W], p)
    left_c = np.cumsum(pl, axis=-1)
    y_full[:, :, :PAD1] = left_c[:, :, W - PAD1 - 1: W - 1]
    pr = np.einsum("gjw,gfjw->gfw", xg[:, :, L - W:], p)
    right_c = np.cumsum(pr[:, :, ::-1], axis=-1)[:, :, ::-1]
    n_right = W - 1 - PAD1
    y_full[:, :, L - n_right:] = right_c[:, :, 1: W - PAD1]
    return np.ascontiguousarray(y_full.reshape(F * L, 1), dtype=np.float32)


def _build_fn(nc):
    """Jitted 8-core shard_map executor for the compiled Bass module.
    Zero-init output buffers are created on device (no host upload)."""
    import jax
    import jax.numpy as jnp
    from jax.sharding import Mesh, PartitionSpec
    from jax.experimental.shard_map import shard_map
    from concourse.bass2jax import (
        _bass_exec_p, install_neuronx_cc_hook, partition_id_tensor)

    install_neuronx_cc_hook()
    partition_name = nc.partition_id_tensor.name if nc.partition_id_tensor else None
    in_names, out_names, out_avals = [], [], []
    for alloc in nc.m.functions[0].allocations:
        if not isinstance(alloc, mybir.MemoryLocationSet):
            continue
        name = alloc.memorylocations[0].name
        if alloc.kind == "ExternalInput":
            if name != partition_name:
                in_names.append(name)
        elif alloc.kind == "ExternalOutput":
            out_names.append(name)
            out_avals.append(jax.core.ShapedArray(
                tuple(alloc.tensor_shape), mybir.dt.np(alloc.dtype)))
    all_names = list(in_names) + list(out_names)
    if partition_name is not None:
        all_names.append(partition_name)

    def _body(*args):
        operands = list(args)
        if partition_name is not None:
            operands.append(partition_id_tensor())
        return tuple(_bass_exec_p.bind(
            *operands,
            out_avals=tuple(out_avals),
            in_names=tuple(all_names),
            out_names=tuple(out_names),
            lowering_input_output_aliases=(),
            sim_require_finite=True,
            sim_require_nnan=True,
            nc=nc,
        ))

    devices = jax.devices()[:NCORES]
    mesh = Mesh(np.asarray(devices), ("core",))
    nin = len(in_names) + len(out_avals)
    fn = jax.jit(shard_map(
        _body, mesh=mesh,
        in_specs=(PartitionSpec("core"),) * nin,
        out_specs=(PartitionSpec("core"),) * len(out_names),
        check_rep=False))
    # zero output buffers, materialized directly on device (no upload)
    sh = jax.sharding.NamedSharding(mesh, PartitionSpec("core"))
    zeros = [
        jax.jit(lambda av=av: jnp.zeros((NCORES * av.shape[0],) + av.shape[1:],
                                        av.dtype), out_shardings=sh)()
        for av in out_avals
    ]
    return fn, in_names, out_names, zeros


def kernel(x, params, rel_idx, _trace=False, _trace_out=None):
    if "nc" not in _cache:
        _cache["nc"] = _build_bass_v5()
        _cache["fn"] = _build_fn(_cache["nc"])
    nc = _cache["nc"]

    xr_cores, wq3, x2, p = _host_prep_v3(x, params, rel_idx)
    try:
        fn, in_names, out_names, zeros = _cache["fn"]
        per = {"xr": np.stack(xr_cores),
               "w": np.broadcast_to(wq3, (NCORES,) + wq3.shape)}
        concat = [np.ascontiguousarray(per[nm].reshape(
            NCORES * per[nm].shape[1], *per[nm].shape[2:])) for nm in in_names]
        outs = fn(*concat, *zeros)
        yi = out_names.index("y")
        y_all = np.asarray(outs[yi]).reshape(NCORES, G, 128, M)
        y_cores = [y_all[c] for c in range(NCORES)]
    except Exception:
        # fallback: reference SPMD runner
        in_maps = [{"xr": xr_cores[c], "w": wq3} for c in range(NCORES)]
        res = run_bass_kernel_spmd(nc, in_maps, list(range(NCORES)))
        y_cores = [np.asarray(res.results[c]["y"]) for c in range(NCORES)]
    return _host_post(y_cores, x2, p)
